# revision 1
# baseline (speedup 1.0000x reference)
"""AttentionClustering kernel for Trainium2, 8 NeuronCores, data-parallel over batch.

Pipeline per core (one image, NCHW f32 in / f32 out):
  conv3x3(replicate pad) + relu  -> conv3x3(replicate pad) + relu -> 1x1 conv
  -> squared-distance logits vs 32 cluster centers -> softmax over clusters
  -> linear recombination with cluster_label.

v2 notes (vs the earlier baseline):
  * The PE never reached 2.4 GHz in the baseline: frequent sub-us stalls
    (PSUM drain waits, dup-DMA on the critical path) kept HAM throttled at
    1.2 GHz for the whole run.  v2 software-pipelines conv1 one strip ahead
    so the q1 shifted-copy DMA has a full strip to land, interleaves
    independent accumulation chains at different PE column groups (measured
    109 ns/MM for col-tiled pairs vs 216 ns serial), and pushes the softmax
    tail 1-2 strips behind the conv stream so PE never waits on ACT/DVE.
  * Softmax denominators stay on-chip: sum via ones-matmul, reciprocal on
    DVE straight out of PSUM, then a K=4 ones-matmul broadcasts 1/d back to
    all 128 partitions (replaces the DRAM round-trip + gpsimd broadcast).
  * Outputs of one strip-group pair leave in a single DMA via a multi-dim
    AP (partition dims [2,64]); xcol im2col loads are 3 DMAs (one per dy).
  * A short warm-up matmul burst at t~9us gets HAM to 2.4 GHz before conv1.
"""
import sys

sys.path.insert(0, "/opt/trn_rl_repo")

import numpy as np
import ml_dtypes

import concourse.bass as bass
import concourse.mybir as mybir
from concourse import bacc, bass_utils
from concourse.tile import TileContext

F32 = mybir.dt.float32
F16 = mybir.dt.float16
BF16 = mybir.dt.bfloat16

B, CIN, H, W = 8, 3, 256, 256
Q, NC, COUT = 64, 32, 64
R = 16          # output rows per strip
S = H // R      # strips
ACT_F = mybir.ActivationFunctionType
ALU = mybir.AluOpType

_cache = {}


def _build():
    nc = bacc.Bacc()
    xpad_t = nc.dram_tensor("xpad", (CIN, H + 2, W + 2), F16, kind="ExternalInput")
    w1c_t = nc.dram_tensor("w1c", (27, Q), F16, kind="ExternalInput")
    wa_t = nc.dram_tensor("wa", (128, 384), F16, kind="ExternalInput")
    wc_t = nc.dram_tensor("wc", (128, 128), F16, kind="ExternalInput")
    mu2b_t = nc.dram_tensor("mu2b", (128, NC), F16, kind="ExternalInput")
    lb4_t = nc.dram_tensor("lb4", (128, COUT), BF16, kind="ExternalInput")
    ones_t = nc.dram_tensor("onesb", (128, 4), BF16, kind="ExternalInput")
    bc8_t = nc.dram_tensor("bc8", (36, 128), BF16, kind="ExternalInput")
    b1_t = nc.dram_tensor("b1c", (Q, 1), F32, kind="ExternalInput")
    b2_t = nc.dram_tensor("b2c", (128, 1), F32, kind="ExternalInput")
    nmun_t = nc.dram_tensor("nmun", (128, 1), F32, kind="ExternalInput")
    out_t = nc.dram_tensor("res", (COUT, H, W), F32, kind="ExternalOutput")

    with TileContext(nc) as tc:
        with (
            tc.tile_pool(name="consts", bufs=1) as cpool,
            tc.tile_pool(name="xcol", bufs=3) as xcol_pool,
            tc.tile_pool(name="q1p", bufs=2) as q1_pool,
            tc.tile_pool(name="q1c", bufs=2) as q1c_pool,
            tc.tile_pool(name="q2", bufs=3) as q2_pool,
            tc.tile_pool(name="e4", bufs=8) as e_pool,
            tc.tile_pool(name="e4s", bufs=4) as es_pool,
            tc.tile_pool(name="rec", bufs=4) as rec_pool,
            tc.tile_pool(name="resf", bufs=4) as res_pool,
            tc.tile_pool(name="c23", bufs=3, space="PSUM") as psum_c23,
            tc.tile_pool(name="psb", bufs=1, space="PSUM") as psum_sb,
        ):
            w1cT = cpool.tile([27, Q], F16)
            nc.sync.dma_start(w1cT[:, :], w1c_t[:, :])
            waT = cpool.tile([128, 384], F16)
            nc.sync.dma_start(waT[:, :], wa_t[:, :])
            wcT = cpool.tile([128, 128], F16)
            nc.sync.dma_start(wcT[:, :], wc_t[:, :])
            mu2T = cpool.tile([128, NC], F16)
            nc.sync.dma_start(mu2T[:, :], mu2b_t[:, :])
            lb4T = cpool.tile([128, COUT], BF16)
            nc.sync.dma_start(lb4T[:, :], lb4_t[:, :])
            onesT = cpool.tile([128, 4], BF16)
            nc.sync.dma_start(onesT[:, :], ones_t[:, :])
            bc8T = cpool.tile([36, 128], BF16)
            nc.sync.dma_start(bc8T[:, :], bc8_t[:, :])
            b1T = cpool.tile([Q, 1], F32)
            nc.scalar.dma_start(b1T[:, :], b1_t[:, :])
            b2T = cpool.tile([128, 1], F32)
            nc.scalar.dma_start(b2T[:, :], b2_t[:, :])
            nmunT = cpool.tile([128, 1], F32)
            nc.scalar.dma_start(nmunT[:, :], nmun_t[:, :])

            e4s, e4ss, recs = {}, {}, {}

            # ---------------- stage helpers (strip u = data index) -------
            def sb_logits(u):
                """logits + exp for strip u, direct from q2 (conv3 folded in)."""
                qt = q2ts.pop(u)
                for Qd in range(2):
                    ps = psum_sb.tile([128, 512], F32, tag="ps", bufs=3)
                    for j in range(4):
                        g = 4 * Qd + j
                        h = 64 * (g % 2)
                        nc.tensor.matmul(
                            ps[32 * j : 32 * j + 32, :],
                            mu2T[h : h + 64, :],
                            qt[h : h + 64, 512 * (g // 2) : 512 * (g // 2 + 1)],
                            start=True, stop=True, tile_position=(h, 32 * j),
                        )
                    e4 = e_pool.tile([128, 512], BF16)
                    e4s[(u, Qd)] = e4
                    nc.scalar.activation(e4[:, :], ps[:, :], ACT_F.Exp,
                                         bias=nmunT[:, :], scale=1.0)

            def sb_sum(u):
                """per-pixel denominators for both Qd halves -> one PSUM tile."""
                pd = psum_c23.tile([128, 512], F32, tag="c23")
                for Qd in range(2):
                    nc.tensor.matmul(pd[32 * Qd : 32 * Qd + 4, :], onesT[:, :],
                                     e4s[(u, Qd)][:, :],
                                     start=True, stop=True,
                                     tile_position=(0, 32 * Qd))
                rec = rec_pool.tile([36, 512], BF16)
                recs[u] = rec
                with nc.allow_low_precision(reason="1/d broadcast via PE needs 16-bit; bf16 keeps f32 range"):
                    nc.vector.reciprocal(rec[:, :], pd[0:36, :])

            def sb_bcast(u, Qd):
                """broadcast 1/d to the 4grp x 32cl partition layout, scale e4."""
                rec = recs[u]
                rb = psum_c23.tile([128, 512], F32, tag="c23")
                nc.tensor.matmul(rb[:, :], bc8T[32 * Qd : 32 * Qd + 4, :],
                                 rec[32 * Qd : 32 * Qd + 4, :],
                                 start=True, stop=True,
                                 tile_position=(32 * Qd, 0))
                es = es_pool.tile([128, 512], BF16)
                e4ss[(u, Qd)] = es
                with nc.allow_low_precision(reason="normalized attention weights fit bf16"):
                    nc.vector.tensor_tensor(es[:, :], rb[:, :],
                                            e4s.pop((u, Qd))[:, :], op=ALU.mult)
                if Qd == 1:
                    recs.pop(u)

            def sb_label(u, Qd):
                """label recombination (final values) + store."""
                r0 = R * u
                es = e4ss.pop((u, Qd))
                for pp in range(2):
                    pr = psum_sb.tile([128, 512], F32, tag="pr", bufs=2)
                    for k in range(2):
                        j = 2 * pp + k
                        nc.tensor.matmul(
                            pr[64 * k : 64 * k + 64, :],
                            lb4T[32 * j : 32 * j + 32, :],
                            es[32 * j : 32 * j + 32, :],
                            start=True, stop=True,
                            tile_position=(32 * j, 64 * k),
                        )
                    resf = res_pool.tile([128, 512], F32)
                    if pp == 0:
                        nc.vector.tensor_copy(resf[:, :], pr[:, :])
                    else:
                        nc.scalar.activation(resf[:, :], pr[:, :],
                                             ACT_F.Identity, scale=1.0)
                    row = r0 + 8 * Qd + 4 * pp
                    nc.sync.dma_start(
                        out_t[:, row : row + 2, :],
                        resf[0:64, :].rearrange("p (r c) -> p r c", r=2),
                    )
                    nc.scalar.dma_start(
                        out_t[:, row + 2 : row + 4, :],
                        resf[64:128, :].rearrange("p (r c) -> p r c", r=2),
                    )

            xcols = {}

            def load_xcol(s):
                if s == 0:
                    A = 0
                elif s == S - 1:
                    A = H - 18
                else:
                    A = R * s - 1
                xcol = xcol_pool.tile([27, 18, 256], F16)
                xcols[s] = xcol
                dma_eng = [nc.sync, nc.gpsimd, nc.gpsimd]
                for dy in range(3):
                    for dx in range(3):
                        p = (3 * dy + dx) * 3
                        dma_eng[dy].dma_start(
                            xcol[p : p + 3, :, :],
                            xpad_t[:, A + dy : A + dy + 18, dx : dx + 256],
                        )

            q1ps, q1cs = {}, {}

            def conv1(s):
                """conv1 for strip s -> q1p(s), plus pads and the shifted dup."""
                woff = 1 if s == 0 else 0
                q1p = q1_pool.tile([128, 19, 258], F16)
                q1ps[s] = q1p
                xcol = xcols.pop(s)
                for G in range(9):
                    pc1 = psum_c23.tile([64, 512], F32, tag="c23")
                    nc.tensor.matmul(
                        pc1[:, :], w1cT[:, :], xcol[:, 2 * G : 2 * G + 2, :],
                        start=True, stop=True,
                    )
                    a = 2 * G + woff
                    dst = q1p[0:64, a : a + 2, 1:257]
                    src = pc1[:, :].rearrange("p (r c) -> p r c", r=2)
                    if G % 2 == 0:
                        nc.scalar.activation(dst, src, ACT_F.Relu,
                                             bias=b1T[:, :], scale=1.0)
                    else:
                        nc.vector.tensor_scalar(dst, src, b1T[:, :], 0.0,
                                                ALU.add, ALU.max)
                # replicate-pad edges
                if s == 0:
                    nc.vector.tensor_copy(q1p[0:64, 0:1, 1:257],
                                          q1p[0:64, 1:2, 1:257])
                if s == S - 1:
                    nc.vector.tensor_copy(q1p[0:64, 18:19, 1:257],
                                          q1p[0:64, 17:18, 1:257])
                nc.vector.tensor_copy(q1p[0:64, :, 0:1], q1p[0:64, :, 1:2])
                nc.vector.tensor_copy(q1p[0:64, :, 257:258], q1p[0:64, :, 256:257])
                # row-shifted duplicate on partitions 64-127 (dy0/dy1 K-packing)
                nc.sync.dma_start(q1p[64:128, 0:18, :], q1p[0:64, 1:19, :])
                # column-shifted duplicate for the dy2 dx-pair packing
                q1c = q1c_pool.tile([128, 19, 258], F16)
                q1cs[s] = q1c
                nc.scalar.dma_start(q1c[0:64, :, :], q1p[0:64, :, :])
                nc.gpsimd.dma_start(q1c[64:128, :, 0:257], q1p[0:64, :, 1:258])

            def conv2_half(s, pis):
                """conv2 for strip s, pi groups in `pis`; chains interleaved
                across the two col positions."""
                rb = 1 if s == S - 1 else 0
                q1p = q1ps[s]
                q1c = q1cs[s]
                q2t = q2ts[s]
                for pi in pis:
                    pc2 = psum_c23.tile([128, 512], F32, tag="c23")
                    for dx in range(3):
                        for gh in range(2):
                            g = 2 * pi + gh
                            h = 64 * gh
                            nc.tensor.matmul(
                                pc2[h : h + 64, :],
                                waT[:, 64 * dx : 64 * dx + 64],
                                q1p[:, rb + 2 * g : rb + 2 * g + 2, dx : dx + 256],
                                start=(dx == 0), stop=False,
                                tile_position=(0, h),
                            )
                    for gh in range(2):
                        g = 2 * pi + gh
                        h = 64 * gh
                        nc.tensor.matmul(
                            pc2[h : h + 64, :], wcT[:, 0:64],
                            q1c[:, rb + 2 * g + 2 : rb + 2 * g + 4, 0:256],
                            start=False, stop=False, tile_position=(0, h),
                        )
                    for gh in range(2):
                        g = 2 * pi + gh
                        h = 64 * gh
                        nc.tensor.matmul(
                            pc2[h : h + 64, :], wcT[64:128, 64:128],
                            q1c[64:128, rb + 2 * g + 2 : rb + 2 * g + 4, 1:257],
                            start=False, stop=True, tile_position=(64, h),
                        )
                    dst = q2t[:, 512 * pi : 512 * (pi + 1)]
                    if pi % 2 == 0:
                        nc.scalar.activation(dst, pc2[:, :], ACT_F.Relu,
                                             bias=b2T[:, :], scale=1.0)
                    else:
                        nc.vector.tensor_scalar(dst, pc2[:, :], b2T[:, :], 0.0,
                                                ALU.add, ALU.max)

            q2ts = {}

            # ------------------------- main pipeline ---------------------
            # iteration i: conv1(i+1), logits(i-1), conv2(i), sum(i-1),
            #              bcast(i-1), conv3(i), label(i-2)+out
            load_xcol(0)
            load_xcol(1)
            # PE warm-up burst: 10 matmuls on the conv2 weight tile (junk
            # results into a rotating PSUM bank, never read).
            for r in range(10):
                pw = psum_c23.tile([128, 384], F32, tag="c23")
                nc.tensor.matmul(pw[:, :], waT[:, 0:128], waT[:, :],
                                 start=True, stop=True)
            conv1(0)

            for i in range(S):
                if i + 2 < S:
                    load_xcol(i + 2)
                if i + 1 < S:
                    conv1(i + 1)
                q2t_new = q2_pool.tile([128, 2048], F16)
                q2ts[i] = q2t_new
                if i >= 1:
                    sb_logits(i - 1)
                conv2_half(i, (0, 1))
                if i >= 2:
                    sb_sum(i - 2)
                conv2_half(i, (2, 3))
                if i >= 3:
                    sb_bcast(i - 3, 0)
                    sb_bcast(i - 3, 1)
                if i >= 4:
                    sb_label(i - 4, 0)
                    sb_label(i - 4, 1)
                q1ps.pop(i)
                q1cs.pop(i)

            # tail ladder (deps ~3+ calls apart)
            sb_logits(S - 1)
            sb_sum(S - 2)
            sb_bcast(S - 3, 0)
            sb_bcast(S - 3, 1)
            sb_label(S - 4, 0)
            sb_label(S - 4, 1)
            sb_sum(S - 1)
            sb_bcast(S - 2, 0)
            sb_bcast(S - 2, 1)
            sb_label(S - 3, 0)
            sb_label(S - 3, 1)
            sb_bcast(S - 1, 0)
            sb_bcast(S - 1, 1)
            sb_label(S - 2, 0)
            sb_label(S - 2, 1)
            sb_label(S - 1, 0)
            sb_label(S - 1, 1)
    nc.finalize()
    return nc


def _prep_inputs(x, w1, b1, w2, b2, w3, b3, cluster_mu, cluster_label):
    f16 = np.float16
    bf16 = ml_dtypes.bfloat16
    xpad = np.pad(x, ((0, 0), (0, 0), (1, 1), (1, 1)), mode="edge").astype(f16)
    w1c = np.ascontiguousarray(
        w1.transpose(2, 3, 1, 0).reshape(27, Q).astype(f16))
    # wa: [128, 384]; cols 0-191: dy=0 (rows 0-63) / dy=1 (rows 64-127) taps
    #     cols 192-383: dy=2 taps on rows 64-127
    wa = np.zeros((128, 384), f16)
    for dx in range(3):
        wa[0:64, 64 * dx : 64 * dx + 64] = w2[:, :, 0, dx].T
        wa[64:128, 64 * dx : 64 * dx + 64] = w2[:, :, 1, dx].T
        wa[64:128, 192 + 64 * dx : 256 + 64 * dx] = w2[:, :, 2, dx].T
    wc = np.zeros((128, 128), f16)
    wc[0:64, 0:64] = w2[:, :, 2, 0].T
    wc[64:128, 0:64] = w2[:, :, 2, 1].T
    wc[64:128, 64:128] = w2[:, :, 2, 2].T
    mu = cluster_mu.reshape(NC, Q).astype(np.float32)
    m2 = 2.0 * mu
    W3 = w3.reshape(Q, Q).astype(np.float32)
    comb = m2 @ W3                      # (NC, Q): 2mu folded through 1x1 conv
    mu2b = np.ascontiguousarray(np.tile(comb.T.astype(f16), (2, 1)))
    lb4 = np.tile(np.ascontiguousarray(cluster_label.T), (4, 1)).astype(bf16)
    onesb = np.zeros((128, 4), bf16)
    for j in range(4):
        onesb[32 * j : 32 * j + 32, j] = 1
    # bc8: rows 0-3 / 32-35 broadcast one group-denominator row to its
    # 32-partition block: out[p] = rec[p // 32]
    bc8 = np.zeros((36, 128), bf16)
    for r in range(4):
        bc8[r, 32 * r : 32 * r + 32] = 1
        bc8[32 + r, 32 * r : 32 * r + 32] = 1
    mun = np.sum(mu * mu, axis=1) - m2 @ b3.astype(np.float32)
    nmun = np.tile(-mun, 4).reshape(128, 1).astype(np.float32)
    shared = {
        "w1c": w1c, "wa": wa, "wc": wc, "mu2b": mu2b, "lb4": lb4,
        "onesb": onesb, "bc8": bc8,
        "b1c": b1.reshape(Q, 1).astype(np.float32),
        "b2c": np.tile(b2, 2).reshape(128, 1).astype(np.float32),
        "nmun": nmun,
    }
    return [{"xpad": np.ascontiguousarray(xpad[b]), **shared} for b in range(B)]


def run(inputs, trace=False, **trace_kwargs):
    """Build (cached), run on 8 cores, return (output, BassKernelResults)."""
    if "nc" not in _cache:
        _cache["nc"] = _build()
    in_maps = _prep_inputs(**{k: np.asarray(v) for k, v in inputs.items()})
    res = bass_utils.run_bass_kernel_spmd(
        _cache["nc"], in_maps, core_ids=list(range(B)), trace=trace, **trace_kwargs
    )
    out = np.stack([res.results[b]["res"] for b in range(B)]).astype(np.float32)
    return out, res


def kernel(**inputs):
    out, _ = run(inputs)
    return out



# revision 3
# speedup vs baseline: 1.2034x; 1.2034x over previous
"""AttentionClustering kernel for Trainium2, 8 NeuronCores, data-parallel over batch.

Pipeline per core (one image, NCHW f32 in / f32 out):
  conv3x3(replicate pad) + relu  -> conv3x3(replicate pad) + relu -> 1x1 conv
  -> squared-distance logits vs 32 cluster centers -> softmax over clusters
  -> linear recombination with cluster_label.

v3 notes (vs v2):
  * v2 spent ~30 MB/core of SBUF->SBUF DMA building the row-shifted (q1p)
    and col-shifted (q1c) duplicates of the conv1 output; DMA ran at 83%
    occupancy and kept the PE half-clocked.  v3 eliminates both:
    - conv1 emits the row-pair-packed layout directly: two column-group-
      tiled matmuls per 2-row group write one [128,512] PSUM tile whose
      lower half is rows (r, r+1) and upper half rows (r+1, r+2); a single
      activation copies it into q1p, already packed for conv2's K=128
      (dy0,dy1) matmuls.
    - conv2's dy2 taps read q1p directly with K=64 matmuls placed on the
      spare PE row groups (gh0 from the upper half at (64,0), gh1 from the
      lower half at (0,64)), so the col-shifted q1c copy is gone.
  * conv1's im2col is prebuilt on the host into one DRAM tensor holding 4
    identical 32-partition copies (row groups 0/32/64/96), so conv1 runs
    4 concurrent K=32 matmuls; one 1.2 MB DMA per strip replaces v2's 9
    small shifted loads.  NOTE: cycling all four row groups corrupts
    results unless the column-group assignment alternates between pair
    tiles ((0,0),(32,64) then (64,64),(96,0)) - verified on HW.
"""
import sys

sys.path.insert(0, "/opt/trn_rl_repo")

import numpy as np
import ml_dtypes

import concourse.bass as bass
import concourse.mybir as mybir
from concourse import bacc, bass_utils
from concourse.tile import TileContext

F32 = mybir.dt.float32
F16 = mybir.dt.float16
BF16 = mybir.dt.bfloat16

B, CIN, H, W = 8, 3, 256, 256
Q, NC, COUT = 64, 32, 64
R = 16          # output rows per strip
S = H // R      # strips
ACT_F = mybir.ActivationFunctionType
ALU = mybir.AluOpType

_cache = {}


def _build():
    nc = bacc.Bacc()
    xcold_t = nc.dram_tensor("xcold", (128, H, W), F16, kind="ExternalInput")
    w1c4_t = nc.dram_tensor("w1c4", (128, Q), F16, kind="ExternalInput")
    wa_t = nc.dram_tensor("wa", (128, 192), F16, kind="ExternalInput")
    wdy2_t = nc.dram_tensor("wdy2", (128, 192), F16, kind="ExternalInput")
    mu2b_t = nc.dram_tensor("mu2b", (128, NC), F16, kind="ExternalInput")
    lb4_t = nc.dram_tensor("lb4", (128, COUT), BF16, kind="ExternalInput")
    ones_t = nc.dram_tensor("onesb", (128, 4), BF16, kind="ExternalInput")
    bc8_t = nc.dram_tensor("bc8", (36, 128), BF16, kind="ExternalInput")
    b1_t = nc.dram_tensor("b1c", (128, 1), F32, kind="ExternalInput")
    b2_t = nc.dram_tensor("b2c", (128, 1), F32, kind="ExternalInput")
    nmun_t = nc.dram_tensor("nmun", (128, 1), F32, kind="ExternalInput")
    out_t = nc.dram_tensor("res", (COUT, H, W), F32, kind="ExternalOutput")

    with TileContext(nc) as tc:
        with (
            tc.tile_pool(name="consts", bufs=1) as cpool,
            tc.tile_pool(name="xcol", bufs=3) as xcol_pool,
            tc.tile_pool(name="q1p", bufs=2) as q1_pool,
            tc.tile_pool(name="q2", bufs=3) as q2_pool,
            tc.tile_pool(name="e4", bufs=8) as e_pool,
            tc.tile_pool(name="e4s", bufs=4) as es_pool,
            tc.tile_pool(name="rec", bufs=4) as rec_pool,
            tc.tile_pool(name="resf", bufs=4) as res_pool,
            tc.tile_pool(name="c23", bufs=3, space="PSUM") as psum_c23,
            tc.tile_pool(name="psb", bufs=1, space="PSUM") as psum_sb,
        ):
            w1c4T = cpool.tile([128, Q], F16)
            nc.sync.dma_start(w1c4T[:, :], w1c4_t[:, :])
            waT = cpool.tile([128, 192], F16)
            nc.sync.dma_start(waT[:, :], wa_t[:, :])
            wdy2T = cpool.tile([128, 192], F16)
            nc.sync.dma_start(wdy2T[:, :], wdy2_t[:, :])
            mu2T = cpool.tile([128, NC], F16)
            nc.sync.dma_start(mu2T[:, :], mu2b_t[:, :])
            lb4T = cpool.tile([128, COUT], BF16)
            nc.sync.dma_start(lb4T[:, :], lb4_t[:, :])
            onesT = cpool.tile([128, 4], BF16)
            nc.sync.dma_start(onesT[:, :], ones_t[:, :])
            bc8T = cpool.tile([36, 128], BF16)
            nc.sync.dma_start(bc8T[:, :], bc8_t[:, :])
            b1T = cpool.tile([128, 1], F32)
            nc.scalar.dma_start(b1T[:, :], b1_t[:, :])
            b2T = cpool.tile([128, 1], F32)
            nc.scalar.dma_start(b2T[:, :], b2_t[:, :])
            nmunT = cpool.tile([128, 1], F32)
            nc.scalar.dma_start(nmunT[:, :], nmun_t[:, :])

            e4s, e4ss, recs = {}, {}, {}

            # ---------------- softmax/label stages (as v2) ---------------
            def sb_logits(u):
                """logits + exp for strip u, direct from q2 (conv3 folded in)."""
                qt = q2ts.pop(u)
                for Qd in range(2):
                    ps = psum_sb.tile([128, 512], F32, tag="ps", bufs=3)
                    for j in range(4):
                        g = 4 * Qd + j
                        h = 64 * (g % 2)
                        nc.tensor.matmul(
                            ps[32 * j : 32 * j + 32, :],
                            mu2T[h : h + 64, :],
                            qt[h : h + 64, 512 * (g // 2) : 512 * (g // 2 + 1)],
                            start=True, stop=True, tile_position=(h, 32 * j),
                        )
                    e4 = e_pool.tile([128, 512], BF16)
                    e4s[(u, Qd)] = e4
                    nc.scalar.activation(e4[:, :], ps[:, :], ACT_F.Exp,
                                         bias=nmunT[:, :], scale=1.0)

            def sb_sum(u):
                """per-pixel denominators for both Qd halves -> one PSUM tile."""
                pd = psum_c23.tile([128, 512], F32, tag="c23")
                for Qd in range(2):
                    nc.tensor.matmul(pd[32 * Qd : 32 * Qd + 4, :], onesT[:, :],
                                     e4s[(u, Qd)][:, :],
                                     start=True, stop=True,
                                     tile_position=(0, 32 * Qd))
                rec = rec_pool.tile([36, 512], BF16)
                recs[u] = rec
                with nc.allow_low_precision(reason="1/d broadcast via PE needs 16-bit; bf16 keeps f32 range"):
                    nc.vector.reciprocal(rec[:, :], pd[0:36, :])

            def sb_bcast(u, Qd):
                """broadcast 1/d to the 4grp x 32cl partition layout, scale e4."""
                rec = recs[u]
                rb = psum_c23.tile([128, 512], F32, tag="c23")
                nc.tensor.matmul(rb[:, :], bc8T[32 * Qd : 32 * Qd + 4, :],
                                 rec[32 * Qd : 32 * Qd + 4, :],
                                 start=True, stop=True,
                                 tile_position=(32 * Qd, 0))
                es = es_pool.tile([128, 512], BF16)
                e4ss[(u, Qd)] = es
                with nc.allow_low_precision(reason="normalized attention weights fit bf16"):
                    nc.vector.tensor_tensor(es[:, :], rb[:, :],
                                            e4s.pop((u, Qd))[:, :], op=ALU.mult)
                if Qd == 1:
                    recs.pop(u)

            def sb_label(u, Qd):
                """label recombination (final values) + store."""
                r0 = R * u
                es = e4ss.pop((u, Qd))
                for pp in range(2):
                    pr = psum_sb.tile([128, 512], F32, tag="pr", bufs=2)
                    for k in range(2):
                        j = 2 * pp + k
                        nc.tensor.matmul(
                            pr[64 * k : 64 * k + 64, :],
                            lb4T[32 * j : 32 * j + 32, :],
                            es[32 * j : 32 * j + 32, :],
                            start=True, stop=True,
                            tile_position=(32 * j, 64 * k),
                        )
                    resf = res_pool.tile([128, 512], F32)
                    if pp == 0:
                        nc.vector.tensor_copy(resf[:, :], pr[:, :])
                    else:
                        nc.scalar.activation(resf[:, :], pr[:, :],
                                             ACT_F.Identity, scale=1.0)
                    row = r0 + 8 * Qd + 4 * pp
                    nc.sync.dma_start(
                        out_t[:, row : row + 2, :],
                        resf[0:64, :].rearrange("p (r c) -> p r c", r=2),
                    )
                    nc.scalar.dma_start(
                        out_t[:, row + 2 : row + 4, :],
                        resf[64:128, :].rearrange("p (r c) -> p r c", r=2),
                    )

            # ---------------- conv1: packed-PSUM scheme ------------------
            xcols = {}

            def strip_lo(s):
                return max(0, R * s - 1)

            def load_xcol(s):
                lo = strip_lo(s)
                hi = min(H - 1, R * s + 17)
                xcol = xcol_pool.tile([128, 19, 256], F16)
                xcols[s] = xcol
                nc.sync.dma_start(xcol[:, 0 : hi - lo + 1, :],
                                  xcold_t[:, lo : hi + 1, :])

            q1ps = {}

            # col-group assignment per G parity (HW-verified safe pattern):
            #   G even: A=(0,0)  B=(32,64);  G odd: B=(64,64) A=(96,0)
            def conv1(s):
                Y0 = R * s
                lo = strip_lo(s)
                xcol = xcols.pop(s)
                q1p = q1_pool.tile([128, 18, 258], F16)
                q1ps[s] = q1p
                for G in range(9):
                    pc1 = psum_c23.tile([128, 512], F32, tag="c23")
                    a0 = Y0 - 1 + 2 * G - lo   # xcol row for MM-A (may be -1)
                    b0 = a0 + 1                # xcol row for MM-B
                    if G % 2 == 0:
                        pA, pB = 0, 32
                    else:
                        pA, pB = 96, 64
                    mms = []
                    if a0 < 0:                 # strip 0, G=0: q1[-1] == q1[0]
                        mms.append((pA, 0, 0, 0, 256))
                        mms.append((pA, 0, 0, 256, 256))
                    elif s == S - 1 and G == 8:  # q1[255], q1[256->255]
                        mms.append((pA, 0, 16, 0, 256))
                        mms.append((pA, 0, 16, 256, 256))
                    else:
                        mms.append((pA, 0, a0, 0, 512))
                    if s == S - 1 and G == 8:   # q1[256->255], junk
                        mms.append((pB, 64, 16, 0, 256))
                        mms.append((pB, 64, 16, 256, 256))
                    else:
                        mms.append((pB, 64, b0, 0, 512))
                    for (rp, cp, row, col, n) in mms:
                        nr = n // 256
                        nc.tensor.matmul(
                            pc1[cp : cp + 64, col : col + n],
                            w1c4T[rp : rp + 32, :],
                            xcol[rp : rp + 32, row : row + nr, :],
                            start=True, stop=True, tile_position=(rp, cp),
                        )
                    dst = q1p[:, 2 * G : 2 * G + 2, 1:257]
                    src = pc1[:, :].rearrange("p (r c) -> p r c", r=2)
                    if G % 2 == 0:
                        nc.scalar.activation(dst, src, ACT_F.Relu,
                                             bias=b1T[:, :], scale=1.0)
                    else:
                        nc.vector.tensor_scalar(dst, src, b1T[:, :], 0.0,
                                                ALU.add, ALU.max)
                # replicate-pad left/right columns
                nc.vector.tensor_copy(q1p[:, :, 0:1], q1p[:, :, 1:2])
                nc.vector.tensor_copy(q1p[:, :, 257:258], q1p[:, :, 256:257])

            # ---------------- conv2: wa K=128 + dy2 K=64 -----------------
            def conv2_half(s, pis):
                q1p = q1ps[s]
                q2t = q2ts[s]
                for pi in pis:
                    pc2 = psum_c23.tile([128, 512], F32, tag="c23")
                    for dx in range(3):
                        for gh in range(2):
                            g = 2 * pi + gh
                            h = 64 * gh
                            nc.tensor.matmul(
                                pc2[h : h + 64, :],
                                waT[:, 64 * dx : 64 * dx + 64],
                                q1p[:, 2 * g : 2 * g + 2, dx : dx + 256],
                                start=(dx == 0), stop=False,
                                tile_position=(0, h),
                            )
                    g0 = 2 * pi
                    g1 = 2 * pi + 1
                    for dx in range(3):
                        nc.tensor.matmul(
                            pc2[0:64, :], wdy2T[64:128, 64 * dx : 64 * dx + 64],
                            q1p[64:128, 2 * g0 + 1 : 2 * g0 + 3, dx : dx + 256],
                            start=False, stop=(dx == 2), tile_position=(64, 0),
                        )
                        nc.tensor.matmul(
                            pc2[64:128, :], wdy2T[0:64, 64 * dx : 64 * dx + 64],
                            q1p[0:64, 2 * g1 + 2 : 2 * g1 + 4, dx : dx + 256],
                            start=False, stop=(dx == 2), tile_position=(0, 64),
                        )
                    dst = q2t[:, 512 * pi : 512 * (pi + 1)]
                    if pi % 2 == 0:
                        nc.scalar.activation(dst, pc2[:, :], ACT_F.Relu,
                                             bias=b2T[:, :], scale=1.0)
                    else:
                        nc.vector.tensor_scalar(dst, pc2[:, :], b2T[:, :], 0.0,
                                                ALU.add, ALU.max)

            q2ts = {}

            # ------------------------- main pipeline ---------------------
            load_xcol(0)
            load_xcol(1)
            # PE warm-up burst (junk results, never read)
            for r in range(10):
                pw = psum_c23.tile([128, 384], F32, tag="c23")
                nc.tensor.matmul(pw[:, 0:192], waT[:, 0:128], waT[:, 0:192],
                                 start=True, stop=True)
            conv1(0)

            for i in range(S):
                if i + 2 < S:
                    load_xcol(i + 2)
                if i + 1 < S:
                    conv1(i + 1)
                q2t_new = q2_pool.tile([128, 2048], F16)
                q2ts[i] = q2t_new
                if i >= 1:
                    sb_logits(i - 1)
                conv2_half(i, (0, 1))
                if i >= 2:
                    sb_sum(i - 2)
                conv2_half(i, (2, 3))
                if i >= 3:
                    sb_bcast(i - 3, 0)
                    sb_bcast(i - 3, 1)
                if i >= 4:
                    sb_label(i - 4, 0)
                    sb_label(i - 4, 1)
                q1ps.pop(i)

            # tail ladder (deps ~3+ calls apart)
            sb_logits(S - 1)
            sb_sum(S - 2)
            sb_bcast(S - 3, 0)
            sb_bcast(S - 3, 1)
            sb_label(S - 4, 0)
            sb_label(S - 4, 1)
            sb_sum(S - 1)
            sb_bcast(S - 2, 0)
            sb_bcast(S - 2, 1)
            sb_label(S - 3, 0)
            sb_label(S - 3, 1)
            sb_bcast(S - 1, 0)
            sb_bcast(S - 1, 1)
            sb_label(S - 2, 0)
            sb_label(S - 2, 1)
            sb_label(S - 1, 0)
            sb_label(S - 1, 1)
    nc.finalize()
    return nc


def _prep_inputs(x, w1, b1, w2, b2, w3, b3, cluster_mu, cluster_label):
    f16 = np.float16
    bf16 = ml_dtypes.bfloat16
    # prebuilt conv1 im2col: 4 identical 32-partition copies (row groups)
    # partition 32b + 3*(3dy+dx) + c = xpad[c, y+dy, x+dx]
    w1c = np.ascontiguousarray(
        w1.transpose(2, 3, 1, 0).reshape(27, Q).astype(f16))
    w1c4 = np.zeros((128, Q), f16)
    for b in range(4):
        w1c4[32 * b : 32 * b + 27] = w1c
    wa = np.zeros((128, 192), f16)
    for dx in range(3):
        wa[0:64, 64 * dx : 64 * dx + 64] = w2[:, :, 0, dx].T
        wa[64:128, 64 * dx : 64 * dx + 64] = w2[:, :, 1, dx].T
    wdy2 = np.zeros((128, 192), f16)
    for dx in range(3):
        wdy2[0:64, 64 * dx : 64 * dx + 64] = w2[:, :, 2, dx].T
        wdy2[64:128, 64 * dx : 64 * dx + 64] = w2[:, :, 2, dx].T
    mu = cluster_mu.reshape(NC, Q).astype(np.float32)
    m2 = 2.0 * mu
    W3 = w3.reshape(Q, Q).astype(np.float32)
    comb = m2 @ W3                      # (NC, Q): 2mu folded through 1x1 conv
    mu2b = np.ascontiguousarray(np.tile(comb.T.astype(f16), (2, 1)))
    lb4 = np.tile(np.ascontiguousarray(cluster_label.T), (4, 1)).astype(bf16)
    onesb = np.zeros((128, 4), bf16)
    for j in range(4):
        onesb[32 * j : 32 * j + 32, j] = 1
    bc8 = np.zeros((36, 128), bf16)
    for r in range(4):
        bc8[r, 32 * r : 32 * r + 32] = 1
        bc8[32 + r, 32 * r : 32 * r + 32] = 1
    mun = np.sum(mu * mu, axis=1) - m2 @ b3.astype(np.float32)
    nmun = np.tile(-mun, 4).reshape(128, 1).astype(np.float32)
    shared = {
        "w1c4": w1c4, "wa": wa, "wdy2": wdy2, "mu2b": mu2b, "lb4": lb4,
        "onesb": onesb, "bc8": bc8,
        "b1c": np.tile(b1, 2).reshape(128, 1).astype(np.float32),
        "b2c": np.tile(b2, 2).reshape(128, 1).astype(np.float32),
        "nmun": nmun,
    }
    xpad = np.pad(x, ((0, 0), (0, 0), (1, 1), (1, 1)), mode="edge").astype(f16)
    maps = []
    for bi in range(B):
        blk = np.zeros((32, H, W), f16)
        for dy in range(3):
            for dx in range(3):
                for c in range(CIN):
                    blk[3 * (3 * dy + dx) + c] = \
                        xpad[bi, c, dy : dy + H, dx : dx + W]
        xcold = np.ascontiguousarray(np.tile(blk, (4, 1, 1)))
        maps.append({"xcold": xcold, **shared})
    return maps


def run(inputs, trace=False, **trace_kwargs):
    """Build (cached), run on 8 cores, return (output, BassKernelResults)."""
    if "nc" not in _cache:
        _cache["nc"] = _build()
    in_maps = _prep_inputs(**{k: np.asarray(v) for k, v in inputs.items()})
    res = bass_utils.run_bass_kernel_spmd(
        _cache["nc"], in_maps, core_ids=list(range(B)), trace=trace, **trace_kwargs
    )
    out = np.stack([res.results[b]["res"] for b in range(B)]).astype(np.float32)
    return out, res


def kernel(**inputs):
    out, _ = run(inputs)
    return out
